# revision 5
# baseline (speedup 1.0000x reference)
"""CEP loss kernel for Trainium2: loss = -sum(d1 * log(d2 + eps)).

Full inputs [4096, 4096] f32 are sharded row-wise across 8 NeuronCores
(512 rows each).  Per core, the shard is streamed as 4 tiles of
[128, 4096]:
  - ScalarE computes L = ln(d2 + eps) in place (the +eps rides the
    activation bias input)
  - VectorE multiplies by d1 in place (fp32 tensor_tensor, 1x)
  - TensorE reduces: ones[128,1].T @ prod[:, j*512:(j+1)*512] matmuls,
    all 32 chunks accumulating into a single PSUM [1, 512] bank
  - tail: PSUM -> SBUF copy, free-dim reduce to [1,1], DMA out
Host sums the 8 per-core scalars and negates.  DMA (~16.8 MB/core @
~358 GB/s HBM limit) is the bottleneck; ACT ~14us, DVE ~17us, PE ~7us
all sit well under the ~47us DMA floor.
"""

import numpy as np

import concourse.bacc as bacc
import concourse.mybir as mybir
import concourse.tile as tile
from concourse.bass_utils import run_bass_kernel_spmd

N = 4096
N_CORES = 8
ROWS_PER_CORE = N // N_CORES  # 512
P = 128
N_TILES = ROWS_PER_CORE // P  # 4
FD = N  # free dim per tile
MM_FD = 512  # one PSUM bank of fp32
N_CHUNKS = FD // MM_FD  # 8
EPS = 1e-5

_NC_CACHE = {}


def _build_nc():
    nc = bacc.Bacc(
        "TRN2", target_bir_lowering=False, debug=False, num_devices=N_CORES
    )
    d1 = nc.dram_tensor(
        "d1", [ROWS_PER_CORE, N], mybir.dt.float32, kind="ExternalInput"
    )
    d2 = nc.dram_tensor(
        "d2", [ROWS_PER_CORE, N], mybir.dt.float32, kind="ExternalInput"
    )
    out = nc.dram_tensor("partial", [1, 1], mybir.dt.float32, kind="ExternalOutput")
    d1t = d1.rearrange("(n p) m -> n p m", p=P)
    d2t = d2.rearrange("(n p) m -> n p m", p=P)

    with tile.TileContext(nc) as tc:
        with (
            tc.tile_pool(name="p1", bufs=3) as p1,
            tc.tile_pool(name="p2", bufs=3) as p2,
            tc.tile_pool(name="paux", bufs=1) as paux,
            tc.tile_pool(name="psum", bufs=1, space="PSUM") as psum_pool,
        ):
            bias = paux.tile([P, 1], mybir.dt.float32)
            nc.vector.memset(bias[:], EPS)
            ones = paux.tile([P, 1], mybir.dt.float32)
            nc.vector.memset(ones[:], 1.0)
            colsum = psum_pool.tile([1, MM_FD], mybir.dt.float32)
            for i in range(N_TILES):
                t1 = p1.tile([P, FD], mybir.dt.float32)
                t2 = p2.tile([P, FD], mybir.dt.float32)
                nc.sync.dma_start(t2[:], d2t[i])
                nc.sync.dma_start(t1[:], d1t[i])
                # t2 <- ln(d2 + eps), in place on ScalarE
                nc.scalar.activation(
                    t2[:], t2[:], mybir.ActivationFunctionType.Ln, bias=bias[:, :]
                )
                # t2 <- d1 * t2, in place on VectorE
                nc.vector.tensor_mul(t2[:], t1[:], t2[:])
                # column-sum chunks into one accumulating PSUM bank
                for j in range(N_CHUNKS):
                    nc.tensor.matmul(
                        colsum[:, :],
                        ones[:, 0:1],
                        t2[:, j * MM_FD : (j + 1) * MM_FD],
                        start=(i == 0 and j == 0),
                        stop=(i == N_TILES - 1 and j == N_CHUNKS - 1),
                    )
            final = paux.tile([1, MM_FD], mybir.dt.float32)
            nc.vector.tensor_copy(final[:], colsum[:])
            total = paux.tile([1, 1], mybir.dt.float32)
            nc.vector.tensor_reduce(
                total[:, 0:1],
                final[:],
                axis=mybir.AxisListType.X,
                op=mybir.AluOpType.add,
            )
            nc.sync.dma_start(out[:], total[:])
    nc.compile()
    return nc


def _get_nc():
    if "nc" not in _NC_CACHE:
        _NC_CACHE["nc"] = _build_nc()
    return _NC_CACHE["nc"]


def run_spmd(in_maps, **kwargs):
    """Run the SPMD kernel; returns BassKernelResults (test harness passes
    trace=True kwargs for profiling)."""
    return run_bass_kernel_spmd(
        _get_nc(), in_maps, core_ids=list(range(N_CORES)), **kwargs
    )


def make_in_maps(distribution1, distribution2):
    d1 = np.asarray(distribution1, dtype=np.float32)
    d2 = np.asarray(distribution2, dtype=np.float32)
    in_maps = []
    for c in range(N_CORES):
        sl = slice(c * ROWS_PER_CORE, (c + 1) * ROWS_PER_CORE)
        in_maps.append(
            {
                "d1": np.ascontiguousarray(d1[sl]),
                "d2": np.ascontiguousarray(d2[sl]),
            }
        )
    return in_maps


def reduce_outputs(results):
    total = np.float64(0.0)
    for r in results:
        total += np.float64(r["partial"].sum(dtype=np.float64))
    return np.asarray([-total], dtype=np.float32)


def kernel(distribution1, distribution2):
    in_maps = make_in_maps(distribution1, distribution2)
    res = run_spmd(in_maps)
    return reduce_outputs(res.results)


# revision 6
# speedup vs baseline: 1.0764x; 1.0764x over previous
"""CEP loss kernel for Trainium2: loss = -sum(d1 * log(d2 + eps)).

Full inputs [4096, 4096] f32 are sharded row-wise across 8 NeuronCores
(512 rows each).  Per core the shard streams as 8 pieces of [128, 2048]
(1 MiB DMAs on the HWDGE queue):
  - ScalarE: t2 <- ln(d2 + eps) in place (+eps rides the activation bias)
  - VectorE: t2 <- d1 * t2 in place (fp32 tensor_tensor, 1x)
  - reduce along free dim to acc[:, piece], alternating engines per piece
    (even: ScalarE activation-Copy accum_out, odd: VectorE tensor_reduce)
    so neither engine becomes the critical path
Each core DMAs its [128, 8] partial-sum tile out; the host sums and
negates.  DMA (~16.8 MB/core at ~360-410 GB/s observed) is the
bottleneck: ACT ~23us and DVE ~26us busy both fit inside the ~41us DMA
window, and the post-DMA tail is only one piece of TT+reduce (~4us).
"""

import numpy as np

import concourse.bacc as bacc
import concourse.mybir as mybir
import concourse.tile as tile
from concourse.bass_utils import run_bass_kernel_spmd

N = 4096
N_CORES = 8
ROWS_PER_CORE = N // N_CORES  # 512
P = 128
N_TILES = ROWS_PER_CORE // P  # 4 row groups
PIECE_FD = 2048
PIECES_PER_TILE = N // PIECE_FD  # 2
N_PIECES = N_TILES * PIECES_PER_TILE  # 8
EPS = 1e-5

_NC_CACHE = {}


def _build_nc():
    nc = bacc.Bacc(
        "TRN2", target_bir_lowering=False, debug=False, num_devices=N_CORES
    )
    d1 = nc.dram_tensor(
        "d1", [ROWS_PER_CORE, N], mybir.dt.float32, kind="ExternalInput"
    )
    d2 = nc.dram_tensor(
        "d2", [ROWS_PER_CORE, N], mybir.dt.float32, kind="ExternalInput"
    )
    out = nc.dram_tensor(
        "partial", [P, N_PIECES], mybir.dt.float32, kind="ExternalOutput"
    )
    d1t = d1.rearrange("(n p) m -> n p m", p=P)
    d2t = d2.rearrange("(n p) m -> n p m", p=P)

    with tile.TileContext(nc) as tc:
        with (
            tc.tile_pool(name="p1", bufs=4) as p1,
            tc.tile_pool(name="p2", bufs=4) as p2,
            tc.tile_pool(name="paux", bufs=1) as paux,
        ):
            bias = paux.tile([P, 1], mybir.dt.float32)
            nc.vector.memset(bias[:], EPS)
            acc = paux.tile([P, N_PIECES], mybir.dt.float32)
            for k in range(N_PIECES):
                i, h = divmod(k, PIECES_PER_TILE)
                fs = slice(h * PIECE_FD, (h + 1) * PIECE_FD)
                t1 = p1.tile([P, PIECE_FD], mybir.dt.float32)
                t2 = p2.tile([P, PIECE_FD], mybir.dt.float32)
                nc.sync.dma_start(t2[:], d2t[i][:, fs])
                nc.sync.dma_start(t1[:], d1t[i][:, fs])
                # t2 <- ln(d2 + eps), in place on ScalarE
                nc.scalar.activation(
                    t2[:], t2[:], mybir.ActivationFunctionType.Ln, bias=bias[:, :]
                )
                # t2 <- d1 * t2, in place on VectorE
                nc.vector.tensor_mul(t2[:], t1[:], t2[:])
                # acc[:, k] = sum_f t2 — alternate reduce engine per piece
                if k % 2 == 0:
                    nc.scalar.activation(
                        t2[:],
                        t2[:],
                        mybir.ActivationFunctionType.Copy,
                        accum_out=acc[:, k : k + 1],
                    )
                else:
                    nc.vector.tensor_reduce(
                        acc[:, k : k + 1],
                        t2[:],
                        axis=mybir.AxisListType.X,
                        op=mybir.AluOpType.add,
                    )
            nc.sync.dma_start(out[:], acc[:])
    nc.compile()
    return nc


def _get_nc():
    if "nc" not in _NC_CACHE:
        _NC_CACHE["nc"] = _build_nc()
    return _NC_CACHE["nc"]


def run_spmd(in_maps, **kwargs):
    """Run the SPMD kernel; returns BassKernelResults (test harness passes
    trace=True kwargs for profiling)."""
    return run_bass_kernel_spmd(
        _get_nc(), in_maps, core_ids=list(range(N_CORES)), **kwargs
    )


def make_in_maps(distribution1, distribution2):
    d1 = np.asarray(distribution1, dtype=np.float32)
    d2 = np.asarray(distribution2, dtype=np.float32)
    in_maps = []
    for c in range(N_CORES):
        sl = slice(c * ROWS_PER_CORE, (c + 1) * ROWS_PER_CORE)
        in_maps.append(
            {
                "d1": np.ascontiguousarray(d1[sl]),
                "d2": np.ascontiguousarray(d2[sl]),
            }
        )
    return in_maps


def reduce_outputs(results):
    total = np.float64(0.0)
    for r in results:
        total += np.float64(r["partial"].sum(dtype=np.float64))
    return np.asarray([-total], dtype=np.float32)


def kernel(distribution1, distribution2):
    in_maps = make_in_maps(distribution1, distribution2)
    res = run_spmd(in_maps)
    return reduce_outputs(res.results)


# revision 8
# speedup vs baseline: 1.1504x; 1.0687x over previous
"""CEP loss kernel for Trainium2: loss = -sum(d1 * log(d2 + eps)).

Full inputs [4096, 4096] f32 are sharded row-wise across 8 NeuronCores
(512 rows each).  Per core the shard streams as 8 pieces of [128, 2048]
(1 MiB DMAs on the HWDGE queue):
  - ScalarE: t2 <- ln(d2 + eps) in place (+eps rides the activation bias)
  - VectorE: t2 <- d1 * t2 in place (fp32 tensor_tensor, 1x)
  - reduce along free dim to acc[:, piece], alternating engines per piece
    (even: ScalarE activation-Copy accum_out, odd: VectorE tensor_reduce)
    so neither engine becomes the critical path
Each core DMAs its [128, 8] partial-sum tile out; the host sums and
negates.  DMA (~16.8 MB/core at ~360-410 GB/s observed) is the
bottleneck: ACT ~23us and DVE ~26us busy both fit inside the ~41us DMA
window, and the post-DMA tail is only one piece of TT+reduce (~4us).
"""

import numpy as np

import concourse.bacc as bacc
import concourse.mybir as mybir
import concourse.tile as tile
from concourse.bass_utils import run_bass_kernel_spmd

N = 4096
N_CORES = 8
ROWS_PER_CORE = N // N_CORES  # 512
P = 128
N_TILES = ROWS_PER_CORE // P  # 4 row groups
PIECE_FD = 2048
PIECES_PER_TILE = N // PIECE_FD  # 2
N_PIECES = N_TILES * PIECES_PER_TILE  # 8
MM_FD = 512  # one PSUM bank of fp32
EPS = 1e-5

_NC_CACHE = {}


def _build_nc():
    nc = bacc.Bacc(
        "TRN2", target_bir_lowering=False, debug=False, num_devices=N_CORES
    )
    d1 = nc.dram_tensor(
        "d1", [ROWS_PER_CORE, N], mybir.dt.float32, kind="ExternalInput"
    )
    d2 = nc.dram_tensor(
        "d2", [ROWS_PER_CORE, N], mybir.dt.float32, kind="ExternalInput"
    )
    out = nc.dram_tensor("partial", [1, 1], mybir.dt.float32, kind="ExternalOutput")
    d1t = d1.rearrange("(n p) m -> n p m", p=P)
    d2t = d2.rearrange("(n p) m -> n p m", p=P)

    with tile.TileContext(nc) as tc:
        with (
            tc.tile_pool(name="p1", bufs=4) as p1,
            tc.tile_pool(name="p2", bufs=4) as p2,
            tc.tile_pool(name="pprod", bufs=4) as pprod,
            tc.tile_pool(name="paux", bufs=1) as paux,
            tc.tile_pool(name="psum", bufs=1, space="PSUM") as psum_pool,
        ):
            bias = paux.tile([P, 1], mybir.dt.float32)
            nc.vector.memset(bias[:], EPS)
            ones = paux.tile([P, 1], mybir.dt.bfloat16)
            nc.vector.memset(ones[:], 1.0)
            colsum = psum_pool.tile([1, MM_FD], mybir.dt.float32)
            for k in range(N_PIECES):
                i, h = divmod(k, PIECES_PER_TILE)
                fs = slice(h * PIECE_FD, (h + 1) * PIECE_FD)
                t1 = p1.tile([P, PIECE_FD], mybir.dt.float32)
                t2 = p2.tile([P, PIECE_FD], mybir.dt.float32)
                prod = pprod.tile([P, PIECE_FD], mybir.dt.bfloat16)
                nc.sync.dma_start(t2[:], d2t[i][:, fs])
                nc.sync.dma_start(t1[:], d1t[i][:, fs])
                # t2 <- ln(d2 + eps), in place on ScalarE
                nc.scalar.activation(
                    t2[:], t2[:], mybir.ActivationFunctionType.Ln, bias=bias[:, :]
                )
                # prod <- d1 * t2 on VectorE, cast to bf16 on the write
                nc.vector.tensor_mul(prod[:], t1[:], t2[:])
                # column sums on the otherwise-idle TensorE (native bf16
                # matmul), every chunk accumulating into one PSUM bank
                for j in range(PIECE_FD // MM_FD):
                    nc.tensor.matmul(
                        colsum[:, :],
                        ones[:, 0:1],
                        prod[:, j * MM_FD : (j + 1) * MM_FD],
                        start=(k == 0 and j == 0),
                        stop=(k == N_PIECES - 1 and j == PIECE_FD // MM_FD - 1),
                    )
            total = paux.tile([1, 1], mybir.dt.float32)
            nc.vector.tensor_reduce(
                total[:, 0:1],
                colsum[:],
                axis=mybir.AxisListType.X,
                op=mybir.AluOpType.add,
            )
            nc.sync.dma_start(out[:], total[:])
    nc.compile()
    return nc


def _get_nc():
    if "nc" not in _NC_CACHE:
        _NC_CACHE["nc"] = _build_nc()
    return _NC_CACHE["nc"]


def run_spmd(in_maps, **kwargs):
    """Run the SPMD kernel; returns BassKernelResults (test harness passes
    trace=True kwargs for profiling)."""
    return run_bass_kernel_spmd(
        _get_nc(), in_maps, core_ids=list(range(N_CORES)), **kwargs
    )


def make_in_maps(distribution1, distribution2):
    d1 = np.asarray(distribution1, dtype=np.float32)
    d2 = np.asarray(distribution2, dtype=np.float32)
    in_maps = []
    for c in range(N_CORES):
        sl = slice(c * ROWS_PER_CORE, (c + 1) * ROWS_PER_CORE)
        in_maps.append(
            {
                "d1": np.ascontiguousarray(d1[sl]),
                "d2": np.ascontiguousarray(d2[sl]),
            }
        )
    return in_maps


def reduce_outputs(results):
    total = np.float64(0.0)
    for r in results:
        total += np.float64(r["partial"].sum(dtype=np.float64))
    return np.asarray([-total], dtype=np.float32)


def kernel(distribution1, distribution2):
    in_maps = make_in_maps(distribution1, distribution2)
    res = run_spmd(in_maps)
    return reduce_outputs(res.results)
